# revision 24
# baseline (speedup 1.0000x reference)
"""Multi-head causal attention with RoPE on 8 Trainium2 NeuronCores.

Sharding: 2 (batch) x 4 (head-groups of 4 heads). Each core computes
QKV projections, RoPE, flash-style causal attention and its slice of the
output projection for one batch and 4 heads; partial outputs are summed
on the host (row-sharded out_proj => partial-sum reduction).

Device layout choices (everything host-prepped to avoid on-device
transposes, fp32 has no DMA-transpose path):
  - x is passed pre-transposed per batch: xT [D, S] bf16
  - Q^T, K^T computed as [head_dim, S] (lhsT = W tile, rhs = xT)
  - V computed natural [S, head_dim] (lhsT = xT tile, rhs = Wv)
  - scores computed transposed [k, q]; softmax sum over k (partitions)
    via a full-width all-ones stationary matmul, which lands the same sum
    on every PSUM partition so normalization needs no broadcast
  - RoPE rotate-half via two SBUF->SBUF partition-swap DMAs with the
    rotation sign folded into the host-prepped sin table (no PE work)
  - attention g-outer; the out-proj tile of q-group g-1 is emitted inside
    head h of group g, between the score matmuls and the PV matmuls, so
    the PE chews on out-proj work exactly where it used to stall waiting
    for the Exp activation; scores pipelined one k-pair ahead of PV
"""

import math
import sys

import numpy as np

try:
    import concourse.bass as bass  # noqa: F401
except Exception:
    sys.path.insert(0, "/opt/trn_rl_repo")

import ml_dtypes

P = 128
B = 2
S = 2048
D = 2048
H = 16
HEAD = 128
N_CORES = 8
HG = 4            # head groups (tensor-parallel dimension)
HPG = H // HG     # heads per group = 4
DG = HPG * HEAD   # group width = 512
SG = 512          # q-group (free dim) size
DOUT = 2048

BF16 = ml_dtypes.bfloat16


def _emit(tc, io, cfg, sfx=""):
    """Emit the per-core program. io: dict of dram APs. cfg: sizes."""
    import concourse.mybir as mybir

    nc = tc.nc
    bf = mybir.dt.bfloat16
    f32 = mybir.dt.float32
    Exp = mybir.ActivationFunctionType.Exp

    s = cfg["S"]
    d = cfg["D"]
    dout = cfg["DOUT"]
    di_t = d // P          # d_in k-tiles
    st = s // P            # seq 128-tiles
    nsg = s // SG          # seq 512-groups
    nos = dout // SG       # out column slices
    inv_sqrt_hd = 1.0 / math.sqrt(HEAD)

    xT = io["xT"].rearrange("(o p) s -> p o s", p=P)
    wq = io["wq"].rearrange("(o p) n -> p o n", p=P)
    wk = io["wk"].rearrange("(o p) n -> p o n", p=P)
    wv = io["wv"].rearrange("(o p) n -> p o n", p=P)
    wo = io["wo"].rearrange("(o p) n -> p o n", p=P)

    const = tc.alloc_tile_pool(name="const" + sfx, bufs=1)
    stores = tc.alloc_tile_pool(name="stores" + sfx, bufs=1)
    # crossover pools live across the phase 1 -> 2 boundary; a scoped
    # `with` exit here would flush the whole group-0 attention chain and
    # stall the PE ~7.5 us at the transition
    xover = tc.alloc_tile_pool(name="xover" + sfx, bufs=4)
    xrec = tc.alloc_tile_pool(name="xrec" + sfx, bufs=1)
    ps_main = tc.alloc_tile_pool(name="ps_main" + sfx, bufs=3, space="PSUM")
    ps2 = tc.alloc_tile_pool(name="ps2" + sfx, bufs=2, space="PSUM")
    ps_sum = tc.alloc_tile_pool(name="ps_sum" + sfx, bufs=1, space="PSUM")

    # ---- constants (tiles only; DMAs emitted after the xT stream) ----
    cos_sb = const.tile([P, s], bf, tag="cos")
    sin_sb = const.tile([P, s], bf, tag="sin")   # sign-folded rotate-half sin
    mask_sb = const.tile([P, HG, SG], bf, tag="mask")
    ones_bf_sb = const.tile([P, P], bf, tag="ones_bf")

    # persistent activation stores
    qt_sb = stores.tile([P, HPG, s], bf, tag="qt")
    kt_sb = stores.tile([P, HPG, s], bf, tag="kt")
    v_sb = stores.tile([P, st, DG], bf, tag="v")
    ctx_sb = stores.tile([P, HPG, s], bf, tag="ctx")

    def emit_scores(g, h, pool):
        """Paired score MMs + exp per [P, 2*SG]; DVE pre-reduces each pair.
        Diagonal pairs (the only masked ones, whose exp->mask chain is
        longest) go FIRST so the mask-mul never gates the tail of the
        head. Returns (ats, dsums) for the PV stage."""
        qsl = slice(g * SG, (g + 1) * SG)
        jmax = min((g + 1) * SG // P, st)
        diag = g * SG // P
        order = [j for j in range(diag, jmax, 2)] + \
                [j for j in range(0, diag, 2)]
        ats = []
        dsums = []
        for j in order:
            ps2t = ps2.tile([P, 2, SG], f32, tag="ps2")
            nc.tensor.matmul(
                ps2t[:, 0, :],
                lhsT=kt_sb[:, h, j * P:(j + 1) * P],
                rhs=qt_sb[:, h, qsl],
                start=True,
                stop=True,
            )
            nc.tensor.matmul(
                ps2t[:, 1, :],
                lhsT=kt_sb[:, h, (j + 1) * P:(j + 2) * P],
                rhs=qt_sb[:, h, qsl],
                start=True,
                stop=True,
            )
            at2 = pool.tile([P, 2, SG], bf, tag="at")
            nc.scalar.activation(at2, ps2t, Exp, scale=inv_sqrt_hd)
            r = j - diag
            if r >= 0:
                nc.vector.tensor_mul(at2, at2, mask_sb[:, r:r + 2, :])
            ats.append((j, at2))
            dsum = pool.tile([P, SG], bf, tag="dsum")
            nc.vector.tensor_add(dsum, at2[:, 0, :], at2[:, 1, :])
            dsums.append(dsum)
        return ats, dsums

    def emit_pv(g, h, ats, dsums, pool, rpool):
        qsl = slice(g * SG, (g + 1) * SG)
        pctx = ps_main.tile([P, SG], f32, tag="ps")
        psum_l = ps_sum.tile([P, SG], f32, tag="l")
        for idx, (j, at2) in enumerate(ats):
            for jj in range(2):
                nc.tensor.matmul(
                    pctx,
                    lhsT=v_sb[:, j + jj, h * P:(h + 1) * P],
                    rhs=at2[:, jj, :],
                    start=(idx == 0 and jj == 0),
                    stop=(idx == len(ats) - 1 and jj == 1),
                )
        # full DVE pair-sum tree, then one softmax-sum matmul
        lvl = dsums
        while len(lvl) > 1:
            nxt = []
            for i in range(0, len(lvl), 2):
                if i + 1 < len(lvl):
                    d2 = pool.tile([P, SG], bf, tag="dsum2")
                    nc.vector.tensor_add(d2, lvl[i], lvl[i + 1])
                    nxt.append(d2)
                else:
                    nxt.append(lvl[i])
            lvl = nxt
        nc.tensor.matmul(psum_l, lhsT=ones_bf_sb[:], rhs=lvl[0],
                         start=True, stop=True)
        rec = rpool.tile([P, SG], f32, tag="rec")
        nc.vector.reciprocal_approx_fast(rec, psum_l)
        nc.vector.tensor_mul(ctx_sb[:, h, qsl], pctx, rec)

    # ---- phase 1: projections + RoPE ----
    with tc.tile_pool(name="xt" + sfx, bufs=1) as xtp, \
         tc.tile_pool(name="wqk" + sfx, bufs=2) as wqkp, \
         tc.tile_pool(name="p1tmp" + sfx, bufs=4) as p1tmp, \
         tc.tile_pool(name="rotp" + sfx, bufs=3) as rotp:
        xt_sb = xtp.tile([P, di_t, s], bf, tag="xt")
        # ones first (feeds the PE warmup), then wv + xT in 128-col chunks
        # (si-major) so V si-tiles stream as soon as their columns land.
        nc.sync.dma_start(ones_bf_sb[:], io["ones_bf"][:])
        nc.sync.dma_start(cos_sb[:], io["cosT"][:])
        nc.sync.dma_start(sin_sb[:], io["sinT"][:])
        nc.sync.dma_start(mask_sb[:], io["masks"][:])
        with tc.tile_pool(name="wvp" + sfx, bufs=1) as wvp:
            wv_sb = wvp.tile([P, di_t, DG], bf, tag="wv")
            for o in range(di_t):
                nc.sync.dma_start(wv_sb[:, o, :], wv[:, o, :])
                nc.sync.dma_start(xt_sb[:, o, 0:P], xT[:, o, 0:P])
            for o in range(di_t):
                nc.sync.dma_start(xt_sb[:, o, P:SG], xT[:, o, P:SG])
            for g in range(1, nsg):
                for o in range(di_t):
                    nc.sync.dma_start(
                        xt_sb[:, o, g * SG:(g + 1) * SG],
                        xT[:, o, g * SG:(g + 1) * SG]
                    )

            # PE warmup: keep the tensor engine streaming through its
            # p-state ramp while the xT DMAs land, so the first real
            # matmuls run at full clock. Fed from a memset tile so it has
            # no DMA dependency.
            wsrc = p1tmp.tile([P, P], bf, tag="wsrc")
            nc.vector.memset(wsrc[:], 0.0)
            warm = ps_sum.tile([P, SG], f32, tag="l")
            for _ in range(32):
                nc.tensor.matmul(warm[:, 0:64], lhsT=wsrc[:],
                                 rhs=wsrc[:, 0:64], start=True, stop=True)

            # V natural layout: [s_tile, DG]
            for si in range(st):
                pv = ps_main.tile([P, SG], f32, tag="ps")
                for o in range(di_t):
                    nc.tensor.matmul(
                        pv[:, :DG],
                        lhsT=xt_sb[:, o, si * P:(si + 1) * P],
                        rhs=wv_sb[:, o, :],
                        start=(o == 0),
                        stop=(o == di_t - 1),
                    )
                nc.vector.tensor_copy(v_sb[:, si, :], pv[:, :DG])

        # Q^T, K^T with RoPE per head. rotate-half = partition swap (DMA)
        # with the sign folded into sin_sb; the DVE mul/add pipeline is
        # deferred a few steps so the swap DMAs complete off critical path.
        # After each head's rope, its (g=0) score matmuls + exps are
        # emitted — the exps drain behind the next head's projections — and
        # the PREVIOUS head's (g=0) PV runs, by which point its exps are
        # long done. Group 0's attention thus costs no exp-wait bubbles.
        if True:
            g0s = {}
            for h in range(HPG):
                wq_t = wqkp.tile([P, di_t, P], bf, tag="wq")
                wk_t = wqkp.tile([P, di_t, P], bf, tag="wk")
                for o in range(di_t):
                    nc.sync.dma_start(wq_t[:, o, :], wq[:, o, h * P:(h + 1) * P])
                    nc.sync.dma_start(wk_t[:, o, :], wk[:, o, h * P:(h + 1) * P])

                def emit_rope(qa, rq, dst, hh, sl):
                    t1 = p1tmp.tile([P, SG], bf, tag="t1")
                    nc.vector.tensor_mul(t1, qa, cos_sb[:, sl])
                    t2 = p1tmp.tile([P, SG], bf, tag="t2")
                    nc.vector.tensor_mul(t2, rq, sin_sb[:, sl])
                    nc.vector.tensor_add(dst[:, hh, sl], t1, t2)

                pending = []
                for g in range(nsg):
                    sl = slice(g * SG, (g + 1) * SG)
                    for w_t, dst in ((wq_t, qt_sb), (wk_t, kt_sb)):
                        pq = ps_main.tile([P, SG], f32, tag="ps")
                        for o in range(di_t):
                            nc.tensor.matmul(
                                pq,
                                lhsT=w_t[:, o, :],
                                rhs=xt_sb[:, o, sl],
                                start=(o == 0),
                                stop=(o == di_t - 1),
                            )
                        qa = p1tmp.tile([P, SG], bf, tag="qa")
                        nc.scalar.copy(qa, pq)
                        rq = rotp.tile([P, SG], bf, tag="rq")
                        nc.sync.dma_start(rq[0:64, :], qa[64:128, :])
                        nc.sync.dma_start(rq[64:128, :], qa[0:64, :])
                        pending.append((qa, rq, dst, h, sl))
                        while len(pending) > 3:
                            emit_rope(*pending.pop(0))
                    if h == HPG - 1 and g == 1:
                        # last head: its crossover must be emitted BEFORE
                        # the remaining rope writes exist, or its
                        # (coarsened) wait covers the whole rope tail and
                        # stalls the PE ~8 us at the phase boundary
                        while pending:
                            emit_rope(*pending.pop(0))
                        g0s[h] = emit_scores(0, h, xover)
                        emit_pv(0, h - 1, *g0s[h - 1], xover, xrec)
                while pending:
                    emit_rope(*pending.pop(0))
                if h < HPG - 1:
                    g0s[h] = emit_scores(0, h, xover)
                    if h > 0:
                        emit_pv(0, h - 1, *g0s[h - 1], xover, xrec)
            emit_pv(0, HPG - 1, *g0s[HPG - 1], xover, xrec)

    # ---- phase 2+3: attention with out-proj stuffed into exp-wait slots ----
    # Group 0 was already handled inside phase 1. Per (g, h): score
    # matmuls first, then the out-proj tile of q-group g-1 (16 matmuls)
    # runs while the Exp activations drain, then PV.
    with tc.tile_pool(name="p2tmp" + sfx, bufs=10) as p2tmp, \
         tc.tile_pool(name="p2rb" + sfx, bufs=3) as p2rb, \
         tc.tile_pool(name="outp" + sfx, bufs=3) as outp, \
         tc.tile_pool(name="wop" + sfx, bufs=1) as wop:
        wo_sb = wop.tile([P, HPG, dout], bf, tag="wo")
        for o in range(HPG):
            nc.sync.dma_start(wo_sb[:, o, :], wo[:, o, :])

        def emit_outproj(qt):
            for dsl in range(nos):
                po = ps_main.tile([P, SG], f32, tag="ps")
                for h2 in range(HPG):
                    nc.tensor.matmul(
                        po,
                        lhsT=ctx_sb[:, h2, qt * P:(qt + 1) * P],
                        rhs=wo_sb[:, h2, dsl * SG:(dsl + 1) * SG],
                        start=(h2 == 0),
                        stop=(h2 == HPG - 1),
                    )
                ob = outp.tile([P, SG], bf, tag="ob")
                nc.vector.tensor_copy(ob, po)
                nc.sync.dma_start(
                    io["out"][qt * P:(qt + 1) * P, dsl * SG:(dsl + 1) * SG], ob
                )

        for g in range(1, nsg):
            for h in range(HPG):
                ats, dsums = emit_scores(g, h, p2tmp)
                emit_outproj(4 * (g - 1) + h)
                emit_pv(g, h, ats, dsums, p2tmp, p2rb)

        for h in range(HPG):
            emit_outproj(4 * (nsg - 1) + h)

    for pool in (ps_sum, ps2, ps_main, xrec, xover, stores, const):
        pool.release()


def build_program(cfg=None):
    import concourse.bacc as bacc
    import concourse.mybir as mybir
    import concourse.tile as tile

    cfg = cfg or {"S": S, "D": D, "DOUT": DOUT}
    bf = mybir.dt.bfloat16
    f32 = mybir.dt.float32
    nc = bacc.Bacc()
    io = {
        "xT": nc.dram_tensor("xT", [cfg["D"], cfg["S"]], bf, kind="ExternalInput"),
        "wq": nc.dram_tensor("wq", [cfg["D"], DG], bf, kind="ExternalInput"),
        "wk": nc.dram_tensor("wk", [cfg["D"], DG], bf, kind="ExternalInput"),
        "wv": nc.dram_tensor("wv", [cfg["D"], DG], bf, kind="ExternalInput"),
        "wo": nc.dram_tensor("wo", [DG, cfg["DOUT"]], bf, kind="ExternalInput"),
        "cosT": nc.dram_tensor("cosT", [P, cfg["S"]], bf, kind="ExternalInput"),
        "sinT": nc.dram_tensor("sinT", [P, cfg["S"]], bf, kind="ExternalInput"),
        "masks": nc.dram_tensor("masks", [P, HG, SG], bf, kind="ExternalInput"),
        "ones_bf": nc.dram_tensor("ones_bf", [P, P], bf, kind="ExternalInput"),
        "out": nc.dram_tensor(
            "out", [cfg["S"], cfg["DOUT"]], bf, kind="ExternalOutput"
        ),
    }
    with tile.TileContext(nc) as tc:
        for rep in range(cfg.get("repeat", 1)):
            _emit(tc, io, cfg, sfx=f"_r{rep}")
    nc.finalize()
    return nc


def host_constants(s=S):
    inv = 1.0 / (10000.0 ** (np.arange(0, HEAD, 2, dtype=np.float32) / HEAD))
    pos = np.arange(s, dtype=np.float32)
    ang = pos[:, None] * inv[None, :]
    ang = np.concatenate([ang, ang], axis=-1)          # (s, HEAD)
    cosT = np.cos(ang).T.astype(BF16).copy()           # (HEAD, s)
    sinT = np.sin(ang).T.astype(np.float32)
    sinT[:64] *= -1.0                                  # rotate-half sign fold
    sinT = sinT.astype(BF16).copy()
    kk = np.arange(P)[:, None, None]
    rr = np.arange(HG)[None, :, None]
    qq = np.arange(SG)[None, None, :]
    masks = (kk <= qq - P * rr).astype(BF16)           # (P, HG, SG)
    ones_bf = np.ones((P, P), BF16)
    return cosT, sinT, masks, ones_bf


def make_in_maps(x, W_query, W_key, W_value, W_out):
    cosT, sinT, masks, ones_bf = host_constants()
    xTb = [np.ascontiguousarray(np.asarray(x[b]).T).astype(BF16) for b in range(B)]
    in_maps = []
    for core in range(N_CORES):
        b, g = divmod(core, HG)
        gsl = slice(g * DG, (g + 1) * DG)
        in_maps.append({
            "xT": xTb[b],
            "wq": np.asarray(W_query)[:, gsl].astype(BF16).copy(),
            "wk": np.asarray(W_key)[:, gsl].astype(BF16).copy(),
            "wv": np.asarray(W_value)[:, gsl].astype(BF16).copy(),
            "wo": np.asarray(W_out)[gsl, :].astype(BF16).copy(),
            "cosT": cosT, "sinT": sinT, "masks": masks, "ones_bf": ones_bf,
        })
    return in_maps


def kernel(x, W_query, W_key, W_value, W_out):
    from concourse.bass_utils import run_bass_kernel_spmd

    x = np.asarray(x)
    in_dtype = x.dtype
    nc = build_program()
    in_maps = make_in_maps(x, W_query, W_key, W_value, W_out)

    res = run_bass_kernel_spmd(nc, in_maps, core_ids=list(range(N_CORES)))
    out = np.zeros((B, S, DOUT), np.float32)
    for core in range(N_CORES):
        b = core // HG
        out[b] += res.results[core]["out"].astype(np.float32)
    return out.astype(in_dtype, copy=False)
